# revision 1
# baseline (speedup 1.0000x reference)
"""Trainium2 Bass kernel for LogHarmonicLowering.

out[b, k*C + c, j, t] = wv0[k,j] * x[b, c, j+d_k, t] + wv1[k,j] * x[b, c, j+d_k+1, t]

with zero padding past the frequency range. The bilinear shift per k has a
constant integer part d_k plus per-(k,j) float32 weights wv0/wv1 precomputed
on host with the exact float32 arithmetic of the reference's grid method.

Distribution: data-parallel over batch — 8 cores, one batch element each.

Per-core scheme: partition dim = frequency. Compute-engine SBUF operands must
start at partition 0/32/64/96, so partition-shifted operand reads are illegal.
Instead:
  - the +1 row shift is materialized at load time: X1 = x shifted down one
    frequency row (DMA reads have no partition-offset restriction),
  - the integer shift d_k is applied at store time via the store DMA's
    SBUF-side partition offset.
Everything on ACT/DVE is then a full-tile partition-aligned op in INPUT row
space i (output row j = i - d):
  ACT:  tmp[i] = wv1[k, i-d] * X1[i]         (Copy activation, per-row scale)
  DVE:  Z[i]   = (X[i] * wv0[k, i-d]) + tmp[i]   (scalar_tensor_tensor)
  store out[., i-d, .] = Z[i]  for i in [d, 256)
k = 0 (shift 0) is an exact copy: input tiles are stored straight back.
Trailing d zero rows come from a persistent zeroed tile.
"""

import functools

import numpy as np

import concourse.bass as bass
import concourse.mybir as mybir
from concourse.bass_utils import run_bass_kernel_spmd
from concourse.tile import TileContext

FK = 5
ANCHOR = 1
OUT_LOG = 12.0
IN_LOG = 1.0
RADIX = 2.0

B, C, F, T = 8, 32, 256, 512
N_CORES = 8


def _host_weights(Fr):
    """Per-(k, j) bilinear weights, float32 ops matching the jax reference."""
    np_shift = (np.arange(FK) + 1) / ANCHOR
    ls = OUT_LOG * np.log(IN_LOG * np_shift) / np.log(RADIX)
    ls -= ls[ANCHOR - 1]
    ls32 = ls.astype(np.float32)
    shift_px = ls32 * np.float32(Fr / (Fr - 1))
    y = np.arange(Fr, dtype=np.float32)[None, :] + shift_px[:, None]
    y0f = np.floor(y)
    w1 = y - y0f
    w0 = np.float32(1.0) - w1
    y0 = y0f.astype(np.int32)
    y1 = y0 + 1
    v0 = ((y0 >= 0) & (y0 < Fr)).astype(np.float32)
    v1 = ((y1 >= 0) & (y1 < Fr)).astype(np.float32)
    wv0 = w0 * v0
    wv1 = w1 * v1
    d = y0[:, 0]
    # the integer shift is constant along j (fractional parts never round
    # across an integer boundary in f32 for these shifts)
    assert (y0 == d[:, None] + np.arange(Fr, dtype=np.int32)[None, :]).all()
    return wv0, wv1, d


def build_nc(C=C, Fr=F, T=T, G=4, NBUF=3, TBUF=2, ZBUF=3):
    """Raw-bass per-core program: x[C,Fr,T] -> out[FK*C,Fr,T].

    Hand-scheduled pipeline (this toolchain's walrus allows only ONE sync
    wait per instruction, so Tile's attached multi-waits cannot compile;
    every wait here is its own wait_ge instruction).
      SP : all DMAs (one HWDGE ring -> count-based completion waits are safe)
      ACT: tmp = wv1 * X1 (Copy activation, per-partition scale)
      DVE: Z = (X * wv0) + tmp (scalar_tensor_tensor), zeros memset
    """
    from contextlib import ExitStack

    wv0, wv1, dks = _host_weights(Fr)
    H = Fr // 2
    nG = C // G
    f32 = mybir.dt.float32
    dmax = int(dks.max())

    ncols = 2 * (FK - 1) * 2

    def col(a, ki, t):
        return (a * (FK - 1) + ki) * 2 + t

    wvtab = np.zeros((H, ncols), np.float32)
    idx = np.arange(H)
    for ki in range(FK - 1):
        k = ki + 1
        d = int(dks[k])
        for a, wv in ((0, wv0), (1, wv1)):
            av = np.where(idx >= d, wv[k, np.maximum(idx - d, 0)], np.float32(0))
            wvtab[:, col(a, ki, 0)] = av
            wvtab[:, col(a, ki, 1)] = wv[k, idx + H - d]

    Copy = mybir.ActivationFunctionType.Copy
    mult = mybir.AluOpType.mult
    add = mybir.AluOpType.add

    nc = bass.Bass(trn_type="TRN2")
    x_h = nc.dram_tensor("x", [C, Fr, T], f32, kind="ExternalInput")
    out_h = nc.dram_tensor("out", [FK * C, Fr, T], f32, kind="ExternalOutput")
    wv_h = nc.inline_tensor(wvtab, name="wvtab")

    def dram(ap):
        return ap.rearrange("c f t -> f c t")

    # op-count bookkeeping (1-based)
    def act_after(g, k):          # s_act value once tmpB(g,k) done
        return 8 * g + 2 * k + 2

    def dve_after(g, k):          # s_dve value once ZB(g,k) done (memset=#1)
        return 8 * g + 2 * k + 3

    def store_idx(g, k=None, which=0):
        # cumulative SP store count: per group 2 k0 + 3 per k
        base = 14 * g
        if k is None:
            return base + 2
        return base + 2 + 3 * k + which  # which: 1=ZA, 2=ZB, 3=zeros

    load_after = {}  # g -> cumulative load-DMA count once group g loaded

    with ExitStack() as ctx:
        sb = lambda shape, name: ctx.enter_context(
            nc.sbuf_tensor(name, shape, f32))
        wvt = sb([H, ncols], "wvt")
        zeros = sb([dmax, G, T], "zeros")
        XA = [sb([H, G, T], f"XA{s}") for s in range(NBUF)]
        XB = [sb([H, G, T], f"XB{s}") for s in range(NBUF)]
        X1A = [sb([H, G, T], f"X1A{s}") for s in range(NBUF)]
        X1B = [sb([H, G, T], f"X1B{s}") for s in range(NBUF)]
        tA = [sb([H, G, T], f"tA{s}") for s in range(TBUF)]
        tB = [sb([H, G, T], f"tB{s}") for s in range(TBUF)]
        ZA = [sb([H, G, T], f"ZA{s}") for s in range(ZBUF)]
        ZB = [sb([H, G, T], f"ZB{s}") for s in range(ZBUF)]
        sem = lambda name: ctx.enter_context(nc.semaphore(name))
        s_wv = sem("s_wv")
        s_ld = [sem(f"s_ld{s}") for s in range(NBUF)]
        s_xst = [sem(f"s_xst{s}") for s in range(NBUF)]
        s_zst = [sem(f"s_zst{s}") for s in range(ZBUF)]
        s_zr = sem("s_zr")
        s_act = sem("s_act")
        s_dve = sem("s_dve")
        block = ctx.enter_context(nc.Block())

        class W:  # monotone wait elision per engine
            def __init__(self, e):
                self.e, self.seen = e, {}
            def __call__(self, sem_, v):
                if v > self.seen.get(id(sem_), 0):
                    self.e.wait_ge(sem_, v)
                    self.seen[id(sem_)] = v

        nzr = [0]

        @block.sync
        def _(e):
            w = W(e)
            e.dma_start(out=wvt[:, :], in_=wv_h[:, :]).then_inc(s_wv, 16)

            def issue_loads(g):
                s = g % NBUF
                u = g // NBUF
                w(s_ld[s], 80 * u)          # own-sem order for detector
                xg = x_h[g * G:(g + 1) * G, :, :]
                e.dma_start(out=XA[s][:, :, :], in_=dram(xg[:, 0:H, :])).then_inc(s_ld[s], 16)
                e.dma_start(out=XB[s][:, :, :], in_=dram(xg[:, H:Fr, :])).then_inc(s_ld[s], 16)
                e.dma_start(out=X1A[s][:, :, :], in_=dram(xg[:, 1:H + 1, :])).then_inc(s_ld[s], 16)
                e.dma_start(out=X1B[s][0:H - 1, :, :], in_=dram(xg[:, H + 1:Fr, :])).then_inc(s_ld[s], 16)
                e.dma_start(out=X1B[s][H - 1:H, :, :], in_=dram(xg[:, Fr - 1:Fr, :])).then_inc(s_ld[s], 16)

            for g in range(min(NBUF, nG)):
                issue_loads(g)
            for g in range(nG):
                s = g % NBUF
                u = g // NBUF
                og0 = out_h[g * G:(g + 1) * G, :, :]
                w(s_ld[s], 80 * (u + 1))
                w(s_xst[s], 32 * u)
                e.dma_start(out=dram(og0[:, 0:H, :]), in_=XA[s][:, :, :]).then_inc(s_xst[s], 16)
                e.dma_start(out=dram(og0[:, H:Fr, :]), in_=XB[s][:, :, :]).then_inc(s_xst[s], 16)
                for k in range(FK - 1):
                    d = int(dks[k + 1])
                    i = 4 * g + k
                    z = i % ZBUF
                    uz = i // ZBUF
                    og = out_h[(k + 1) * C + g * G:(k + 1) * C + (g + 1) * G, :, :]
                    w(s_zst[z], 32 * uz)
                    w(s_dve, dve_after(g, k) - 1)   # ZA ready
                    e.dma_start(out=dram(og[:, 0:H - d, :]), in_=ZA[z][d:H, :, :]).then_inc(s_zst[z], 16)
                    w(s_dve, dve_after(g, k))       # ZB ready
                    e.dma_start(out=dram(og[:, H - d:Fr - d, :]), in_=ZB[z][:, :, :]).then_inc(s_zst[z], 16)
                    w(s_zr, 16 * max(0, nzr[0] - 8))
                    e.dma_start(out=dram(og[:, Fr - d:Fr, :]), in_=zeros[0:d, :, :]).then_inc(s_zr, 16)
                    nzr[0] += 1
                gn = g + NBUF
                if gn < nG:
                    # recycle slot: ACT/DVE consumed group g, k0 stores landed
                    w(s_act, act_after(g, FK - 2))
                    w(s_dve, dve_after(g, FK - 2))
                    w(s_xst[s], 32 * (u + 1))
                    issue_loads(gn)
            # drain every DMA sem before program end
            w(s_zr, 16 * nzr[0])
            for z in range(ZBUF):
                uses = sum(1 for i in range(4 * nG) if i % ZBUF == z)
                w(s_zst[z], 32 * uses)
            for s in range(NBUF):
                uses = sum(1 for g in range(nG) if g % NBUF == s)
                w(s_xst[s], 32 * uses)

        @block.scalar
        def _(e):
            w = W(e)
            w(s_wv, 16)
            for g in range(nG):
                s = g % NBUF
                u = g // NBUF
                for k in range(FK - 1):
                    i = 4 * g + k
                    t = i % TBUF
                    w(s_ld[s], 80 * (u + 1))
                    if i >= TBUF:  # tmp slot recycle: reader stt of pair i-TBUF
                        g2, k2 = divmod(i - TBUF, 4)
                        w(s_dve, dve_after(g2, k2))
                    e.activation(tA[t][:, :, :], X1A[s][:, :, :], Copy,
                                 scale=wvt[:, col(1, k, 0):col(1, k, 0) + 1]
                                 ).then_inc(s_act, 1)
                    e.activation(tB[t][:, :, :], X1B[s][:, :, :], Copy,
                                 scale=wvt[:, col(1, k, 1):col(1, k, 1) + 1]
                                 ).then_inc(s_act, 1)

        @block.vector
        def _(e):
            w = W(e)
            e.memset(zeros[:, :, :], 0.0).then_inc(s_dve, 1)
            w(s_wv, 16)
            for g in range(nG):
                s = g % NBUF
                u = g // NBUF
                for k in range(FK - 1):
                    i = 4 * g + k
                    t = i % TBUF
                    z = i % ZBUF
                    uz = i // ZBUF
                    w(s_act, act_after(g, k))
                    w(s_ld[s], 80 * (u + 1))
                    w(s_zst[z], 32 * uz)   # Z slot recycle: prior stores done
                    e.scalar_tensor_tensor(
                        ZA[z][:, :, :], XA[s][:, :, :],
                        wvt[:, col(0, k, 0):col(0, k, 0) + 1],
                        tA[t][:, :, :], mult, add).then_inc(s_dve, 1)
                    e.scalar_tensor_tensor(
                        ZB[z][:, :, :], XB[s][:, :, :],
                        wvt[:, col(0, k, 1):col(0, k, 1) + 1],
                        tB[t][:, :, :], mult, add).then_inc(s_dve, 1)
    return nc


@functools.lru_cache(maxsize=1)
def _get_nc():
    return build_nc()


def _run(x, trace=False):
    in_maps = [{"x": np.ascontiguousarray(x[b])} for b in range(B)]
    res = run_bass_kernel_spmd(_get_nc(), in_maps, core_ids=list(range(N_CORES)),
                               trace=trace)
    out = np.stack([r["out"] for r in res.results], axis=0)
    return out, res


def kernel(x):
    x = np.asarray(x)
    assert x.shape == (B, C, F, T), x.shape
    out, _ = _run(x)
    return out



# revision 8
# speedup vs baseline: 260.2296x; 260.2296x over previous
"""Trainium2 Bass kernel for LogHarmonicLowering.

out[b, k*C + c, j, t] = wv0[k,j] * x[b, c, j+d_k, t] + wv1[k,j] * x[b, c, j+d_k+1, t]

with zero padding past the frequency range. The bilinear shift per k has a
constant integer part d_k plus per-(k,j) float32 weights wv0/wv1 precomputed
on host with the exact float32 arithmetic of the reference's grid method.

Distribution: data-parallel over batch — 8 cores, one batch element each.

Per-core scheme (v5, tuned against axon-tunnelled TRN2 measurements):
  - partition dim = frequency (f), tiles [128, G, T] f32.
  - ACT engine (2nd HWDGE ring) issues ALL loads; SP issues ALL stores.
    Mixing reads+writes on one ring measured ~2.5x slower than splitting.
  - Every DMA covers a full 128-partition range: partial-partition DMAs
    (e.g. 127-row shifted loads, d-row zero stores) measured 4-10x slower.
    The +1-row-shifted X1B tiles are loaded per channel from a flat
    (c f) t view so the 128th row wraps into the next channel (its value
    is killed by a zero weight). Z stores are per-channel flat stores
    whose first d partitions hold exact zeros (the weight table zeroes
    partitions < d), which lands them on the previous channel's zero
    tail; only the per-shift boundary channel needs a partial store and
    an explicit d-row zero-tail store (4 of each per execution).
  - DVE does the whole bilinear combine as self-synchronised
    scalar_tensor_tensor pairs (tmp = wv1*X1; Z = wv0*X + tmp); measured
    ~0.5us/op, fully hidden under the DMA streams.

reps>1 repeats the whole kernel body back-to-back inside one program
(same DRAM in/out, identical final state). test.py uses the slope of
T(reps) to measure steady-state per-execution device time, cancelling
the ~70ms constant axon-tunnel dispatch latency out of the measurement.
"""

import functools
from contextlib import ExitStack

import numpy as np

import concourse.bass as bass
import concourse.mybir as mybir
from concourse.bass_utils import run_bass_kernel_spmd

FK = 5
ANCHOR = 1
OUT_LOG = 12.0
IN_LOG = 1.0
RADIX = 2.0

B, C, F, T = 8, 32, 256, 512
N_CORES = 8


def _host_weights(Fr):
    """Per-(k, j) bilinear weights, float32 ops matching the jax reference."""
    np_shift = (np.arange(FK) + 1) / ANCHOR
    ls = OUT_LOG * np.log(IN_LOG * np_shift) / np.log(RADIX)
    ls -= ls[ANCHOR - 1]
    ls32 = ls.astype(np.float32)
    shift_px = ls32 * np.float32(Fr / (Fr - 1))
    y = np.arange(Fr, dtype=np.float32)[None, :] + shift_px[:, None]
    y0f = np.floor(y)
    w1 = y - y0f
    w0 = np.float32(1.0) - w1
    y0 = y0f.astype(np.int32)
    y1 = y0 + 1
    v0 = ((y0 >= 0) & (y0 < Fr)).astype(np.float32)
    v1 = ((y1 >= 0) & (y1 < Fr)).astype(np.float32)
    wv0 = w0 * v0
    wv1 = w1 * v1
    d = y0[:, 0]
    # the integer shift is constant along j (fractional parts never round
    # across an integer boundary in f32 for these shifts)
    assert (y0 == d[:, None] + np.arange(Fr, dtype=np.int32)[None, :]).all()
    return wv0, wv1, d


def build_nc(C=C, Fr=F, T=T, G=4, NBUF=3, ZBUF=4, reps=1):
    """Raw-bass per-core program: x[C,Fr,T] -> out[FK*C,Fr,T]."""
    wv0, wv1, dks = _host_weights(Fr)
    H = Fr // 2
    nG = C // G
    nGr = nG * reps
    f32 = mybir.dt.float32
    dmax = int(dks.max())

    ncols = 2 * (FK - 1) * 2

    def col(a, ki, t):
        return (a * (FK - 1) + ki) * 2 + t

    # weight table, one column per (wv0/wv1, k, half); in INPUT row space,
    # with partitions below the integer shift d zeroed (those partitions of
    # ZA then hold exact zeros -> reused as the previous channel's zero tail)
    wvtab = np.zeros((H, ncols), np.float32)
    idx = np.arange(H)
    for ki in range(FK - 1):
        k = ki + 1
        d = int(dks[k])
        for a, wv in ((0, wv0), (1, wv1)):
            av = np.where(idx >= d, wv[k, np.maximum(idx - d, 0)], np.float32(0))
            wvtab[:, col(a, ki, 0)] = av
            wvtab[:, col(a, ki, 1)] = wv[k, idx + H - d]

    mult = mybir.AluOpType.mult
    add = mybir.AluOpType.add
    bypass = mybir.AluOpType.bypass

    nc = bass.Bass(trn_type="TRN2")
    x_h = nc.dram_tensor("x", [C, Fr, T], f32, kind="ExternalInput")
    out_h = nc.dram_tensor("out", [FK * C, Fr, T], f32, kind="ExternalOutput")
    wv_h = nc.inline_tensor(wvtab, name="wvtab")
    xf = x_h.rearrange("c f t -> (c f) t")
    of = out_h.rearrange("c f t -> (c f) t")

    def dram(ap):
        return ap.rearrange("c f t -> f c t")

    def dve_after(vg, k):          # s_dve value once ZB(vg,k) done (memset=#1)
        return 8 * vg + 2 * k + 3

    with ExitStack() as ctx:
        sb = lambda shape, name: ctx.enter_context(
            nc.sbuf_tensor(name, shape, f32))
        wvt = sb([H, ncols], "wvt")
        zeros = sb([dmax, G, T], "zeros")
        XA = [sb([H, G, T], f"XA{s}") for s in range(NBUF)]
        XB = [sb([H, G, T], f"XB{s}") for s in range(NBUF)]
        X1A = [sb([H, G, T], f"X1A{s}") for s in range(NBUF)]
        X1B = [sb([H, G, T], f"X1B{s}") for s in range(NBUF)]
        tA = [sb([H, G, T], f"tA{s}") for s in range(2)]
        tB = [sb([H, G, T], f"tB{s}") for s in range(2)]
        ZA = [sb([H, G, T], f"ZA{s}") for s in range(ZBUF)]
        ZB = [sb([H, G, T], f"ZB{s}") for s in range(ZBUF)]
        sem = lambda name: ctx.enter_context(nc.semaphore(name))
        s_wv = sem("s_wv")
        s_ld = [sem(f"s_ld{s}") for s in range(NBUF)]
        s_xst = [sem(f"s_xst{s}") for s in range(NBUF)]
        s_zst = [sem(f"s_zst{s}") for s in range(ZBUF)]
        NZR = 8
        s_zrs = [sem(f"s_zr{r}") for r in range(NZR)]
        s_tp = sem("s_tp")
        s_dve = sem("s_dve")
        block = ctx.enter_context(nc.Block())

        class W:  # monotone wait elision per engine
            def __init__(self, e):
                self.e, self.seen = e, {}
            def __call__(self, sem_, v):
                if v > self.seen.get(id(sem_), 0):
                    self.e.wait_ge(sem_, v)
                    self.seen[id(sem_)] = v

        nzr = [0]
        # per-slot cumulative s_ld totals: ldsum[s][u] = value before round u
        ldsum = {s: [0] for s in range(NBUF)}

        def issue_loads(e, w, vg):
            s = vg % NBUF
            u = vg // NBUF
            g = vg % nG
            w(s_ld[s], ldsum[s][u])
            if vg >= NBUF:
                pg = vg - NBUF
                pu = pg // NBUF
                w(s_dve, dve_after(pg, FK - 2))   # DVE consumed X of pg
                w(s_xst[s], 32 * (pu + 1))        # k0 stores of pg done
            tot = ldsum[s][u]
            xg = x_h[g * G:(g + 1) * G, :, :]
            e.dma_start(out=XA[s][:, :, :], in_=dram(xg[:, 0:H, :])).then_inc(s_ld[s], 16)
            e.dma_start(out=XB[s][:, :, :], in_=dram(xg[:, H:Fr, :])).then_inc(s_ld[s], 16)
            e.dma_start(out=X1A[s][:, :, :], in_=dram(xg[:, 1:H + 1, :])).then_inc(s_ld[s], 16)
            tot += 48
            for c in range(G):
                cc = g * G + c
                r0 = cc * Fr + H + 1
                if cc < C - 1:
                    # 128 rows; the last wraps into the next channel's row 0
                    # (killed by a zero weight in wvtab)
                    e.dma_start(out=X1B[s][:, c:c + 1, :],
                                in_=xf[r0:r0 + H, :].rearrange("f (o t) -> f o t", o=1)).then_inc(s_ld[s], 16)
                    tot += 16
                else:
                    e.dma_start(out=X1B[s][0:H - 1, c:c + 1, :],
                                in_=xf[r0:r0 + H - 1, :].rearrange("f (o t) -> f o t", o=1)).then_inc(s_ld[s], 16)
                    e.dma_start(out=X1B[s][H - 1:H, c:c + 1, :],
                                in_=xf[r0 - 1:r0, :].rearrange("f (o t) -> f o t", o=1)).then_inc(s_ld[s], 16)
                    tot += 32
            ldsum[s].append(tot)

        @block.scalar
        def _(e):
            w = W(e)
            for vg in range(nGr):
                issue_loads(e, w, vg)

        @block.sync
        def _(e):
            w = W(e)
            e.dma_start(out=wvt[:, :], in_=wv_h[:, :]).then_inc(s_wv, 16)
            for vg in range(nGr):
                s = vg % NBUF
                u = vg // NBUF
                g = vg % nG
                og0 = out_h[g * G:(g + 1) * G, :, :]
                w(s_ld[s], ldsum[s][u + 1])
                w(s_xst[s], 32 * u)
                e.dma_start(out=dram(og0[:, 0:H, :]), in_=XA[s][:, :, :]).then_inc(s_xst[s], 16)
                e.dma_start(out=dram(og0[:, H:Fr, :]), in_=XB[s][:, :, :]).then_inc(s_xst[s], 16)
                for k in range(FK - 1):
                    d = int(dks[k + 1])
                    i = 4 * vg + k
                    z = i % ZBUF
                    uz = i // ZBUF
                    og = out_h[(k + 1) * C + g * G:(k + 1) * C + (g + 1) * G, :, :]
                    w(s_zst[z], 16 * (G + 1) * uz)
                    w(s_dve, dve_after(vg, k) - 1)   # ZA ready
                    for c in range(G):
                        cc = (k + 1) * C + g * G + c
                        if c == 0 and g == 0:
                            # shift-boundary channel: previous channel has a
                            # different d -> no pre-rows, partial store
                            e.dma_start(out=dram(og[0:1, 0:H - d, :]),
                                        in_=ZA[z][d:H, c:c + 1, :]).then_inc(s_zst[z], 16)
                        else:
                            # flat 128-partition store; partitions 0..d are
                            # exact zeros -> previous channel's zero tail
                            r0 = cc * Fr - d
                            e.dma_start(out=of[r0:r0 + H, :].rearrange("f (o t) -> f o t", o=1),
                                        in_=ZA[z][:, c:c + 1, :]).then_inc(s_zst[z], 16)
                    w(s_dve, dve_after(vg, k))       # ZB ready
                    e.dma_start(out=dram(og[:, H - d:Fr - d, :]), in_=ZB[z][:, :, :]).then_inc(s_zst[z], 16)
                    if g == nG - 1:
                        # zero tail of the last channel of this shift
                        mlast = (k + 2) * C - 1
                        rz = mlast * Fr + Fr - d
                        n = nzr[0]
                        if n >= NZR:
                            w(s_zrs[n % NZR], 16 * (n // NZR))
                        e.dma_start(out=of[rz:rz + d, :].rearrange("f (o t) -> f o t", o=1),
                                    in_=zeros[0:d, 0:1, :]).then_inc(s_zrs[n % NZR], 16)
                        nzr[0] += 1
            # drain every DMA sem before program end
            for r in range(NZR):
                uses = sum(1 for n in range(nzr[0]) if n % NZR == r)
                w(s_zrs[r], 16 * uses)
            for z in range(ZBUF):
                uses = sum(1 for i in range(4 * nGr) if i % ZBUF == z)
                w(s_zst[z], 16 * (G + 1) * uses)
            for s in range(NBUF):
                uses = sum(1 for vg in range(nGr) if vg % NBUF == s)
                w(s_xst[s], 32 * uses)

        @block.vector
        def _(e):
            w = W(e)
            e.memset(zeros[:, :, :], 0.0).then_inc(s_dve, 1)
            w(s_wv, 16)
            ntp = [0]
            for vg in range(nGr):
                s = vg % NBUF
                u = vg // NBUF
                for k in range(FK - 1):
                    i = 4 * vg + k
                    t = i % 2
                    z = i % ZBUF
                    uz = i // ZBUF
                    w(s_ld[s], ldsum[s][u + 1])
                    w(s_zst[z], 16 * (G + 1) * uz)   # Z slot recycle
                    if i >= 2:   # WAR: tmp slot reused from pair i-2
                        w(s_dve, dve_after(*divmod(i - 2, 4)))
                    e.scalar_tensor_tensor(
                        tA[t][:, :, :], X1A[s][:, :, :],
                        wvt[:, col(1, k, 0):col(1, k, 0) + 1],
                        X1A[s][:, :, :], mult, bypass).then_inc(s_tp, 1)
                    e.scalar_tensor_tensor(
                        tB[t][:, :, :], X1B[s][:, :, :],
                        wvt[:, col(1, k, 1):col(1, k, 1) + 1],
                        X1B[s][:, :, :], mult, bypass).then_inc(s_tp, 1)
                    ntp[0] += 2
                    w(s_tp, ntp[0])   # self-sync: tmp writes drained
                    e.scalar_tensor_tensor(
                        ZA[z][:, :, :], XA[s][:, :, :],
                        wvt[:, col(0, k, 0):col(0, k, 0) + 1],
                        tA[t][:, :, :], mult, add).then_inc(s_dve, 1)
                    e.scalar_tensor_tensor(
                        ZB[z][:, :, :], XB[s][:, :, :],
                        wvt[:, col(0, k, 1):col(0, k, 1) + 1],
                        tB[t][:, :, :], mult, add).then_inc(s_dve, 1)
    return nc


@functools.lru_cache(maxsize=2)
def _get_nc(reps=1):
    return build_nc(reps=reps)


def _run(x, trace=False):
    in_maps = [{"x": np.ascontiguousarray(x[b])} for b in range(B)]
    res = run_bass_kernel_spmd(_get_nc(), in_maps, core_ids=list(range(N_CORES)),
                               trace=trace)
    out = np.stack([r["out"] for r in res.results], axis=0)
    return out, res


def kernel(x):
    x = np.asarray(x)
    assert x.shape == (B, C, F, T), x.shape
    out, _ = _run(x)
    return out
